# revision 1
# baseline (speedup 1.0000x reference)
"""Causal self-attention (k/q swapped variant) on 8 Trainium2 NeuronCores.

Problem (hardcoded shapes): B=2, N=2048, D=1024, H=16, DH=64.
  kqv = einsum('bnd,hde->bhne', x, Wkqv) + bkqv   ; split -> k, q, v
  A[b,h,n,m] = k[b,h,n]·q[b,h,m] / sqrt(DH), causal mask m<=n, softmax over m
  sa = A @ v ; concat heads ; out = sa @ Wo + bo

Sharding: tensor-parallel over heads — each core owns 2 heads (both batch
entries), computes its partial output projection sa_local @ Wo[rows], and the
host sums the 8 partials (+bo).

Per-core device kernel (all matmul operands bf16, fp32 PSUM accumulation):
  - x is pre-transposed on host to xt[b] = x[b].T ([D, N]) so the contraction
    dim d lands on SBUF partitions. Batch 0 arrives in half-N pieces and its
    k/q projection runs d-chunk-major across 4 concurrent PSUM groups so the
    PE is paced by DMA arrival instead of stalling for the full 4MB.
  - k/q/v projections produce kT/qT/vT in [dh, n] layout with both heads
    stacked on the partition dim; biases are per-partition scalars there.
    v is then rotated to [n, dh] via PE transposes.
  - scores are computed transposed, S^T[m, n] = q[m]·k[n], so softmax's
    reduction dim m sits on partitions; the denominator comes free from the
    PV matmul by augmenting v with 64 ones columns (the psum's other 64 rows
    hold the replicated row-sum). exp() is only computed on the causal
    region; the 128x128 diagonal triangle is zeroed with a 0/1 mask multiply.
  - output projection: stationary = saT column block, moving = Wo rows of the
    local heads -> natural-layout partial out [n, 1024], emitted per j-block
    as soon as its sa rows are final.
  - the whole program is emitted as a fine-grained weave: batch 1's
    projection groups and batch 0's output projection fill the PE gaps of
    batch 0's ACT-bound attention stream, keeping the PE activity monitor
    from re-throttling the clock (HAM) and keeping every engine busy.
"""

import numpy as np
import ml_dtypes

B = 2
N = 2048
D = 1024
H = 16
DH = 64
NCORES = 8
HL = H // NCORES          # heads per core = 2
DC = D // 128             # contraction chunks = 8
NB = N // 128             # 128-row blocks = 16
NJ = N // 512             # 512-col blocks = 4

BF16 = ml_dtypes.bfloat16

_CACHE = {}


def _build():
    import concourse.bass as bass
    import concourse.mybir as mybir
    import concourse.tile as tile
    from concourse import bacc
    from contextlib import ExitStack

    f32 = mybir.dt.float32
    bf16 = mybir.dt.bfloat16
    Exp = mybir.ActivationFunctionType.Exp
    Identity = mybir.ActivationFunctionType.Identity

    nc = bacc.Bacc("TRN2", target_bir_lowering=False, debug=False,
                   enable_asserts=False, num_devices=NCORES)

    xt_d = nc.dram_tensor("xt", [B, D, N], bf16, kind="ExternalInput")
    # k/q/v weights arrive pre-shuffled to the SBUF layout [128, DC*128]
    # (partition = within-chunk row, free = (chunk, head-col))
    wk_d = nc.dram_tensor("wk2", [128, DC * 128], bf16, kind="ExternalInput")
    wq_d = nc.dram_tensor("wq2", [128, DC * 128], bf16, kind="ExternalInput")
    wv_d = nc.dram_tensor("wv2", [128, DC * 128], bf16, kind="ExternalInput")
    wo_d = nc.dram_tensor("wo2", [128, D], bf16, kind="ExternalInput")
    bk_d = nc.dram_tensor("bk2", [128, 1], f32, kind="ExternalInput")
    bq_d = nc.dram_tensor("bq2", [128, 1], f32, kind="ExternalInput")
    bv_d = nc.dram_tensor("bv2", [128, 1], f32, kind="ExternalInput")
    eye_d = nc.dram_tensor("eye2", [128, 64], bf16, kind="ExternalInput")
    m01_d = nc.dram_tensor("m01", [128, 128], bf16, kind="ExternalInput")
    out_d = nc.dram_tensor("out", [B, N, D], f32, kind="ExternalOutput")

    with tile.TileContext(nc) as tc, ExitStack() as ctx:
        const = ctx.enter_context(tc.tile_pool(name="const", bufs=1))
        xt_pool = ctx.enter_context(tc.tile_pool(name="xt", bufs=1))
        kq_pool = ctx.enter_context(tc.tile_pool(name="kq", bufs=6))
        v_pool = ctx.enter_context(tc.tile_pool(name="v", bufs=2))
        sa_pool = ctx.enter_context(tc.tile_pool(name="sa", bufs=2))
        pt_pool = ctx.enter_context(tc.tile_pool(name="pt", bufs=6))
        rc_pool = ctx.enter_context(tc.tile_pool(name="rc", bufs=2))
        ob_pool = ctx.enter_context(tc.tile_pool(name="ob", bufs=6))
        proj_ps = ctx.enter_context(tc.tile_pool(name="proj_ps", bufs=2, space="PSUM"))
        s_ps = ctx.enter_context(tc.tile_pool(name="s_ps", bufs=2, space="PSUM"))
        pv_ps = ctx.enter_context(tc.tile_pool(name="pv_ps", bufs=2, space="PSUM"))
        out_ps = ctx.enter_context(tc.tile_pool(name="out_ps", bufs=2, space="PSUM"))

        # ---- DMA issue order: k/q weights -> xt batch 0 (half-N pieces,
        # alternating HWDGE rings) -> remaining consts -> xt batch 1.
        xt0 = {}   # (dc, half) -> [128, 1024]
        xt1 = {}   # dc -> [128, 2048]

        # weights arrive pre-shuffled from the host as [128, DC*128] so the
        # loads are single contiguous DMAs
        wk_sb = const.tile([128, DC * 128], bf16, name="wk_sb")
        wq_sb = const.tile([128, DC * 128], bf16, name="wq_sb")
        wv_sb = const.tile([128, DC * 128], bf16, name="wv_sb")
        nc.sync.dma_start(wk_sb[:], wk_d.ap())
        nc.sync.dma_start(wq_sb[:], wq_d.ap())
        for half in range(2):
            for dc in range(DC):
                t = xt_pool.tile([128, 1024], bf16, name=f"xt0_{dc}_{half}",
                                 tag="xt0", bufs=2 * DC)
                eng = nc.sync if dc % 2 == 0 else nc.scalar
                eng.dma_start(t[:], xt_d.ap()[0, dc * 128:(dc + 1) * 128,
                                              half * 1024:(half + 1) * 1024])
                xt0[dc, half] = t
        nc.sync.dma_start(wv_sb[:], wv_d.ap())
        wo_sb = const.tile([128, D], bf16, name="wo_sb")
        nc.sync.dma_start(wo_sb[:], wo_d.ap())
        bk_sb = const.tile([128, 1], f32, name="bk_sb")
        nc.sync.dma_start(bk_sb[:], bk_d.ap())
        bq_sb = const.tile([128, 1], f32, name="bq_sb")
        nc.sync.dma_start(bq_sb[:], bq_d.ap())
        bv_sb = const.tile([128, 1], f32, name="bv_sb")
        nc.sync.dma_start(bv_sb[:], bv_d.ap())
        eye_sb = const.tile([128, 64], bf16, name="eye_sb")
        nc.sync.dma_start(eye_sb[:], eye_d.ap())
        m01_sb = const.tile([128, 128], bf16, name="m01_sb")
        nc.sync.dma_start(m01_sb[:], m01_d.ap())
        for dc in range(DC):
            t = xt_pool.tile([128, N], bf16, name=f"xt1_{dc}", tag="xt1",
                             bufs=DC)
            nc.sync.dma_start(t[:], xt_d.ap()[1, dc * 128:(dc + 1) * 128, :])
            xt1[dc] = t

        def xt_ap(b, dc, c0, c1):
            if b == 1:
                return xt1[dc][:, c0:c1]
            half = c0 // 1024
            return xt0[dc, half][:, c0 - half * 1024:c1 - half * 1024]

        # ---- per-batch tensors
        k2 = {b: kq_pool.tile([128, N], bf16, name=f"k2_b{b}", tag="kq")
              for b in range(B)}
        q2 = {b: kq_pool.tile([128, N], bf16, name=f"q2_b{b}", tag="kq")
              for b in range(B)}
        vt = {b: kq_pool.tile([128, N], bf16, name=f"vt_b{b}", tag="kq")
              for b in range(B)}
        v_sb = {}
        sa_sb = {}
        groups = {b: ((wk_sb, bk_sb, k2[b]), (wq_sb, bq_sb, q2[b]),
                      (wv_sb, bv_sb, vt[b])) for b in range(B)}

        def proj_group(b, gi, nj, bias_on_act=False):
            """One [128, 512] projection psum group (k/q/v-T of batch b)."""
            w_sb, bias_sb, dst = groups[b][gi]
            ps = proj_ps.tile([128, 512], f32, name="proj_ps", tag="proj")
            for dc in range(DC):
                nc.tensor.matmul(
                    ps[:], w_sb[:, dc * 128:(dc + 1) * 128],
                    xt_ap(b, dc, nj * 512, (nj + 1) * 512),
                    start=(dc == 0), stop=(dc == DC - 1))
            del bias_on_act  # ACT Identity+bias suspected in HW hang bisect
            nc.vector.tensor_scalar_add(
                dst[:, nj * 512:(nj + 1) * 512], ps[:], bias_sb[:])

        def kq_half0_b0():
            """Batch 0 k/q for nj 0..1, d-chunk-major across 4 concurrent
            groups (paced by the half-0 piece DMAs; borrows out_ps banks)."""
            pss = {}
            for gi in range(2):
                for njl in range(2):
                    pool, tag = (proj_ps, "proj") if gi == 0 else (out_ps, "op")
                    pss[gi, njl] = pool.tile([128, 512], f32, name="kq_ps",
                                             tag=tag)
            for dc in range(DC):
                for gi in range(2):
                    for njl in range(2):
                        nc.tensor.matmul(
                            pss[gi, njl][:],
                            groups[0][gi][0][:, dc * 128:(dc + 1) * 128],
                            xt0[dc, 0][:, njl * 512:(njl + 1) * 512],
                            start=(dc == 0), stop=(dc == DC - 1))
            for gi in range(2):
                _, bias_sb, dst = groups[0][gi]
                for njl in range(2):
                    nc.vector.tensor_scalar_add(
                        dst[:, njl * 512:(njl + 1) * 512], pss[gi, njl][:],
                        bias_sb[:])

        def transpose_v(b, nb):
            """Rotate vT[dh, n] -> v[n, dh] for one 128-row chunk."""
            for h in range(HL):
                tp = proj_ps.tile([128, 64], bf16, name="tp", tag="proj")
                nc.tensor.transpose(
                    tp[:],
                    vt[b][64 * h:64 * h + 64, nb * 128:(nb + 1) * 128],
                    eye_sb[64 * h:64 * h + 64, :])
                nc.vector.tensor_copy(
                    v_sb[b][:, nb * 192 + 128 * h:nb * 192 + 128 * h + 64],
                    tp[:])

        def att_chunk(b, j, ci, pv):
            """Scores + exp + mask + PV accumulate for one 128-m chunk,
            both heads."""
            nch = 4 * (j + 1)
            t = ci - 4 * j
            lo = 128 * t if t >= 0 else 0
            for h in range(HL):
                hp = 64 * h
                sp = s_ps.tile([128, 512], f32, name="s", tag="s")
                nc.tensor.matmul(
                    sp[:, lo:512],
                    q2[b][hp:hp + 64, ci * 128:(ci + 1) * 128],
                    k2[b][hp:hp + 64, j * 512 + lo:(j + 1) * 512],
                    start=True, stop=True)
                pt = pt_pool.tile([128, 512], bf16, name="pt", tag="pt")
                nc.scalar.activation(pt[:, lo:512], sp[:, lo:512], Exp,
                                     scale=0.125)
                if t >= 0:
                    nc.vector.tensor_tensor(
                        pt[:, lo:lo + 128], pt[:, lo:lo + 128],
                        m01_sb[:], mybir.AluOpType.mult)
                nc.tensor.matmul(
                    pv[h][:, lo:512],
                    v_sb[b][:, ci * 192 + 64 * h:ci * 192 + 64 * h + 128],
                    pt[:, lo:512],
                    start=(ci == 0), stop=(ci == nch - 1))

        def att_norm(b, j, pv):
            for h in range(HL):
                # h0: psum rows 0:64 = sa, 64:128 = denom ; h1: swapped
                sa_rows = pv[h][64 * h:64 * h + 64, :]
                den_rows = pv[h][64 - 64 * h:128 - 64 * h, :]
                # denominators are sums of exp() in [~2e-3, ~3e3]: inside
                # approx_fast's domain; 18-bit accuracy is far below the bf16
                # noise of the P*V numerator. (approx_fast misreads PSUM
                # operands on HW - bounce through SBUF first.)
                den_sb = rc_pool.tile([64, 512], f32, name="den", tag="den")
                nc.scalar.copy(den_sb[:], den_rows)
                rc = rc_pool.tile([64, 512], f32, name="rc", tag="rc")
                nc.vector.reciprocal_approx_fast(rc[:], den_sb[:])
                nc.vector.tensor_tensor(
                    sa_sb[b][64 * h:64 * h + 64, j * 512:(j + 1) * 512],
                    sa_rows, rc[:], mybir.AluOpType.mult)

        def outproj_block(b, nb):
            for half in range(2):
                op = out_ps.tile([128, 512], f32, name="op", tag="op")
                nc.tensor.matmul(
                    op[:], sa_sb[b][:, nb * 128:(nb + 1) * 128],
                    wo_sb[:, half * 512:(half + 1) * 512],
                    start=True, stop=True)
                ob = ob_pool.tile([128, 512], f32, name="ob", tag="ob")
                nc.vector.tensor_copy(ob[:], op[:])
                nc.sync.dma_start(
                    out_d.ap()[b, nb * 128:(nb + 1) * 128,
                               half * 512:(half + 1) * 512], ob[:])

        def att_j(b, j, weave=()):
            """One attention j-block: new-v transposes, chunk stream (woven
            with filler units), normalize, output-projection of its rows."""
            for nb in range(4 * j, 4 * j + 4):
                transpose_v(b, nb)
            pv = [pv_ps.tile([128, 512], f32, name=f"pv{h}", tag="pv")
                  for h in range(HL)]
            weave = list(weave)
            nch = 4 * (j + 1)
            emitted = 0
            for ci in range(nch):
                att_chunk(b, j, ci, pv)
                target = len(weave) * (ci + 1) // nch
                while emitted < target:
                    weave[emitted]()
                    emitted += 1
            att_norm(b, j, pv)
            for nb in range(4 * j, 4 * j + 4):
                outproj_block(b, nb)

        # ================= emission schedule =================
        for b in range(B):
            v_sb[b] = v_pool.tile([128, NB * 192], bf16, name=f"v_b{b}",
                                  tag="v")
            sa_sb[b] = sa_pool.tile([128, N], bf16, name=f"sa_b{b}", tag="sa")
            nc.vector.memset(
                v_sb[b][:].rearrange("p (nb g) -> p nb g", g=192)[:, :, 64:128],
                1.0)

        kq_half0_b0()                      # b0 k/q nj 0-1 (DMA-paced)
        proj_group(0, 2, 0, bias_on_act=True)   # b0 vT nj 0

        # b0 attention, woven with the rest of b0's projection and all of
        # b1's projection. NOTE: emission order IS dataflow order for Tile —
        # every woven producer must be emitted in an earlier (or the same,
        # pre-consumer) weave slot than its consumer: att_j(b, j)'s
        # transposes read vt nj=j at the START of the j-block, so vt nj=j+1
        # must be woven into block j at the latest.
        att_j(0, 0, weave=[
            lambda: proj_group(0, 2, 1),
            lambda: proj_group(0, 0, 2),
            lambda: proj_group(0, 1, 2),
        ])
        att_j(0, 1, weave=[
            lambda: proj_group(0, 2, 2),
            lambda: proj_group(0, 0, 3),
            lambda: proj_group(0, 1, 3),
            lambda: proj_group(0, 2, 3),
        ])
        att_j(0, 2, weave=[
            lambda: proj_group(1, 0, 0),
            lambda: proj_group(1, 1, 0),
            lambda: proj_group(1, 2, 0),
            lambda: proj_group(1, 0, 1),
            lambda: proj_group(1, 1, 1),
            lambda: proj_group(1, 2, 1),
        ])
        att_j(0, 3, weave=[
            lambda: proj_group(1, 0, 2),
            lambda: proj_group(1, 1, 2),
            lambda: proj_group(1, 2, 2),
            lambda: proj_group(1, 0, 3),
            lambda: proj_group(1, 1, 3),
            lambda: proj_group(1, 2, 3),
        ])
        for j in range(NJ):
            att_j(1, j)

    nc.compile()
    return nc


def _get_nc():
    if "nc" not in _CACHE:
        _CACHE["nc"] = _build()
    return _CACHE["nc"]


def _prep_inputs(x, Wkqv, bkqv, Wo, bo):
    """Host-side shard prep: one input map per core."""
    xt = np.ascontiguousarray(x.transpose(0, 2, 1)).astype(BF16)
    tri = np.triu(np.ones((128, 128), np.float32)).astype(BF16)  # m' <= n''
    eye2 = np.concatenate([np.eye(64, dtype=np.float32)] * 2, axis=0).astype(BF16)
    in_maps = []
    for c in range(NCORES):
        h0, h1 = HL * c, HL * c + 1
        def shuf(w):
            # [D, 128] -> [128, DC*128]: partition = within-chunk row
            return np.ascontiguousarray(
                w.reshape(DC, 128, 128).transpose(1, 0, 2).reshape(128, DC * 128))

        wk2 = shuf(np.concatenate([Wkqv[h0, :, 0:64], Wkqv[h1, :, 0:64]], axis=1))
        wq2 = shuf(np.concatenate([Wkqv[h0, :, 64:128], Wkqv[h1, :, 64:128]], axis=1))
        wv2 = shuf(np.concatenate([Wkqv[h0, :, 128:192], Wkqv[h1, :, 128:192]], axis=1))
        bk2 = np.concatenate([bkqv[h0, 0:64], bkqv[h1, 0:64]])[:, None]
        bq2 = np.concatenate([bkqv[h0, 64:128], bkqv[h1, 64:128]])[:, None]
        bv2 = np.concatenate([bkqv[h0, 128:192], bkqv[h1, 128:192]])[:, None]
        in_maps.append({
            "xt": xt,
            "wk2": wk2.astype(BF16),
            "wq2": wq2.astype(BF16),
            "wv2": wv2.astype(BF16),
            "wo2": Wo[128 * c:128 * (c + 1), :].astype(BF16),
            "bk2": np.ascontiguousarray(bk2, np.float32),
            "bq2": np.ascontiguousarray(bq2, np.float32),
            "bv2": np.ascontiguousarray(bv2, np.float32),
            "eye2": eye2,
            "m01": tri,
        })
    return in_maps


def kernel(x, Wkqv, bkqv, Wo, bo):
    from concourse import bass_utils

    nc = _get_nc()
    in_maps = _prep_inputs(np.asarray(x), np.asarray(Wkqv), np.asarray(bkqv),
                           np.asarray(Wo), np.asarray(bo))
    res = bass_utils.run_bass_kernel_spmd(nc, in_maps, core_ids=list(range(NCORES)))
    acc = np.zeros((B, N, D), np.float32)
    for c in range(NCORES):
        acc += res.results[c]["out"]
    acc += np.asarray(bo)[None, None, :]
    return acc



# revision 5
# speedup vs baseline: 1.0648x; 1.0648x over previous
"""Causal self-attention (k/q swapped variant) on 8 Trainium2 NeuronCores.

Problem (hardcoded shapes): B=2, N=2048, D=1024, H=16, DH=64.
  kqv = einsum('bnd,hde->bhne', x, Wkqv) + bkqv   ; split -> k, q, v
  A[b,h,n,m] = k[b,h,n]·q[b,h,m] / sqrt(DH), causal mask m<=n, softmax over m
  sa = A @ v ; concat heads ; out = sa @ Wo + bo

Sharding: tensor-parallel over heads — each core owns 2 heads (both batch
entries), computes its partial output projection sa_local @ Wo[rows] in bf16,
and the host sums the 8 partials (+bo) in fp32.

Per-core device kernel (all matmul operands bf16, fp32 PSUM accumulation):
  - x is pre-transposed on host to xt[b] = x[b].T ([D, N]) so the contraction
    dim d lands on SBUF partitions.
  - scores are computed transposed, S^T[m, n] = q[m]·k[n], so softmax's
    reduction dim m sits on partitions; both heads' score chunks live in ONE
    [128, 1024] PSUM tile (2 banks) so off-diagonal chunks need a single wide
    exp() on the ACT engine. The denominator comes free from the PV matmul by
    augmenting v with 64 ones columns.
  - the chunk loop is software-pipelined: scores+exp of chunk ci+1 are emitted
    before the PV of chunk ci, so the PE never sits behind the ACT stream;
    projection / output-projection work is woven between chunks in ~0.9us
    slots to fill the remaining PE gaps.
  - elementwise side work (v-rotation copies, causal-mask multiplies, psum
    memsets) runs on the otherwise-idle Pool (gpsimd) engine.
  - PSUM budget (8 banks): scores 2x[128,1024] (4) + PV accumulators 2 +
    shared proj/outproj bank 1 + transpose bank 1.
"""

import numpy as np
import ml_dtypes

B = 2
N = 2048
D = 1024
H = 16
DH = 64
NCORES = 8
HL = H // NCORES          # heads per core = 2
DC = D // 128             # contraction chunks = 8
NB = N // 128             # 128-row blocks = 16
NJ = N // 512             # 512-col blocks = 4

BF16 = ml_dtypes.bfloat16

_CACHE = {}


def _build():
    import concourse.bass as bass
    import concourse.mybir as mybir
    import concourse.tile as tile
    from concourse import bacc
    from contextlib import ExitStack

    f32 = mybir.dt.float32
    bf16 = mybir.dt.bfloat16
    Exp = mybir.ActivationFunctionType.Exp

    nc = bacc.Bacc("TRN2", target_bir_lowering=False, debug=False,
                   enable_asserts=False, num_devices=NCORES)

    xt_d = nc.dram_tensor("xt", [B, D, N], bf16, kind="ExternalInput")
    # k/q/v weights arrive pre-shuffled to the SBUF layout [128, DC*128]
    # (partition = within-chunk row, free = (chunk, head-col))
    wk_d = nc.dram_tensor("wk2", [128, DC * 128], bf16, kind="ExternalInput")
    wq_d = nc.dram_tensor("wq2", [128, DC * 128], bf16, kind="ExternalInput")
    wv_d = nc.dram_tensor("wv2", [128, DC * 128], bf16, kind="ExternalInput")
    wo_d = nc.dram_tensor("wo2", [128, D], bf16, kind="ExternalInput")
    bk_d = nc.dram_tensor("bk2", [128, 1], f32, kind="ExternalInput")
    bq_d = nc.dram_tensor("bq2", [128, 1], f32, kind="ExternalInput")
    bv_d = nc.dram_tensor("bv2", [128, 1], f32, kind="ExternalInput")
    eye_d = nc.dram_tensor("eye2", [128, 128], bf16, kind="ExternalInput")
    m01_d = nc.dram_tensor("m01", [128, 128], bf16, kind="ExternalInput")
    out_d = nc.dram_tensor("out", [B, N, D], bf16, kind="ExternalOutput")

    with tile.TileContext(nc) as tc, ExitStack() as ctx:
        const = ctx.enter_context(tc.tile_pool(name="const", bufs=1))
        xt_pool = ctx.enter_context(tc.tile_pool(name="xt", bufs=1))
        kq_pool = ctx.enter_context(tc.tile_pool(name="kq", bufs=6))
        v_pool = ctx.enter_context(tc.tile_pool(name="v", bufs=2))
        sa_pool = ctx.enter_context(tc.tile_pool(name="sa", bufs=2))
        pt_pool = ctx.enter_context(tc.tile_pool(name="pt", bufs=6))
        rc_pool = ctx.enter_context(tc.tile_pool(name="rc", bufs=2))
        ob_pool = ctx.enter_context(tc.tile_pool(name="ob", bufs=4))
        s_ps = ctx.enter_context(tc.tile_pool(name="s_ps", bufs=2, space="PSUM"))
        pv_ps = ctx.enter_context(tc.tile_pool(name="pv_ps", bufs=2, space="PSUM"))
        wv_ps = ctx.enter_context(tc.tile_pool(name="wv_ps", bufs=1, space="PSUM"))
        tp_ps = ctx.enter_context(tc.tile_pool(name="tp_ps", bufs=1, space="PSUM"))

        # ---- DMA issue order: k/q/v weights -> xt batch 0 (half-N pieces,
        # spread over rings) -> remaining consts -> xt batch 1.
        xt0 = {}   # (dc, half) -> [128, 1024]
        xt1 = {}   # dc -> [128, 2048]

        wk_sb = const.tile([128, DC * 128], bf16, name="wk_sb")
        wq_sb = const.tile([128, DC * 128], bf16, name="wq_sb")
        wv_sb = const.tile([128, DC * 128], bf16, name="wv_sb")
        nc.sync.dma_start(wk_sb[:], wk_d.ap())
        nc.sync.dma_start(wq_sb[:], wq_d.ap())
        nc.sync.dma_start(wv_sb[:], wv_d.ap())
        for half in range(2):
            for dc in range(DC):
                t = xt_pool.tile([128, 1024], bf16, name=f"xt0_{dc}_{half}",
                                 tag="xt0", bufs=2 * DC)
                eng = nc.sync if dc % 2 == 0 else nc.scalar
                eng.dma_start(t[:], xt_d.ap()[0, dc * 128:(dc + 1) * 128,
                                              half * 1024:(half + 1) * 1024])
                xt0[dc, half] = t
        bk_sb = const.tile([128, 1], f32, name="bk_sb")
        nc.sync.dma_start(bk_sb[:], bk_d.ap())
        bq_sb = const.tile([128, 1], f32, name="bq_sb")
        nc.sync.dma_start(bq_sb[:], bq_d.ap())
        bv_sb = const.tile([128, 1], f32, name="bv_sb")
        nc.sync.dma_start(bv_sb[:], bv_d.ap())
        eye_sb = const.tile([128, 128], bf16, name="eye_sb")
        nc.sync.dma_start(eye_sb[:], eye_d.ap())
        m01_sb = const.tile([128, 128], bf16, name="m01_sb")
        nc.sync.dma_start(m01_sb[:], m01_d.ap())
        wo_sb = const.tile([128, D], bf16, name="wo_sb")
        nc.sync.dma_start(wo_sb[:], wo_d.ap())
        for dc in range(DC):
            t = xt_pool.tile([128, N], bf16, name=f"xt1_{dc}", tag="xt1",
                             bufs=DC)
            eng = nc.sync if dc % 2 == 0 else nc.scalar
            eng.dma_start(t[:], xt_d.ap()[1, dc * 128:(dc + 1) * 128, :])
            xt1[dc] = t

        def xt_ap(b, dc, c0, c1):
            if b == 1:
                return xt1[dc][:, c0:c1]
            half = c0 // 1024
            return xt0[dc, half][:, c0 - half * 1024:c1 - half * 1024]

        # ---- per-batch tensors
        k2 = {b: kq_pool.tile([128, N], bf16, name=f"k2_b{b}", tag="kq")
              for b in range(B)}
        q2 = {b: kq_pool.tile([128, N], bf16, name=f"q2_b{b}", tag="kq")
              for b in range(B)}
        vt = {b: kq_pool.tile([128, N], bf16, name=f"vt_b{b}", tag="kq")
              for b in range(B)}
        v_sb = {}
        sa_sb = {}
        groups = {b: ((wk_sb, bk_sb, k2[b]), (wq_sb, bq_sb, q2[b]),
                      (wv_sb, bv_sb, vt[b])) for b in range(B)}

        for b in range(B):
            v_sb[b] = v_pool.tile([128, NB * 192], bf16, name=f"v_b{b}",
                                  tag="v")
            sa_sb[b] = sa_pool.tile([128, N], bf16, name=f"sa_b{b}", tag="sa")
            nc.gpsimd.memset(
                v_sb[b][:].rearrange("p (nb g) -> p nb g", g=192)[:, :, 64:128],
                1.0)

        def make_proj_slots(b, gi, nj):
            """One [128, 512] projection group as two weave slots (dc 0-3 and
            dc 4-7 + bias add), sharing the wv PSUM bank."""
            st = {}

            def part(d0, d1):
                def go():
                    if d0 == 0:
                        st["ps"] = wv_ps.tile([128, 512], f32, name="wvps",
                                              tag="wv")
                    ps = st["ps"]
                    w_sb, bias_sb, dst = groups[b][gi]
                    for dc in range(d0, d1):
                        nc.tensor.matmul(
                            ps[:], w_sb[:, dc * 128:(dc + 1) * 128],
                            xt_ap(b, dc, nj * 512, (nj + 1) * 512),
                            start=(dc == 0), stop=(dc == DC - 1))
                    if d1 == DC:
                        nc.vector.tensor_scalar_add(
                            dst[:, nj * 512:(nj + 1) * 512], ps[:], bias_sb[:])
                return go
            return [part(0, 4), part(4, DC)]

        def make_op_slot(b, nb):
            """Output projection of one 128-row block: 2 half matmuls on the
            shared wv bank -> bf16 SBUF -> one [128, 1024] DMA."""
            def go():
                ob = ob_pool.tile([128, 1024], bf16, name="ob", tag="ob")
                for half in range(2):
                    op = wv_ps.tile([128, 512], f32, name="opps", tag="wv")
                    nc.tensor.matmul(
                        op[:], sa_sb[b][:, nb * 128:(nb + 1) * 128],
                        wo_sb[:, half * 512:(half + 1) * 512],
                        start=True, stop=True)
                    nc.vector.tensor_copy(ob[:, half * 512:(half + 1) * 512],
                                          op[:])
                nc.sync.dma_start(out_d.ap()[b, nb * 128:(nb + 1) * 128, :],
                                  ob[:])
            return go

        def kqv0_start():
            """Batch-0 k/q/v projections for nj=0, d-chunk-major so the PE is
            paced by the half-0 xt piece DMAs (k/q in one score-pool tile)."""
            kq0s = s_ps.tile([128, 1024], f32, name="kq0s", tag="s")
            v0ps = wv_ps.tile([128, 512], f32, name="v0ps", tag="wv")
            for dc in range(DC):
                nc.tensor.matmul(kq0s[:, 0:512],
                                 wk_sb[:, dc * 128:(dc + 1) * 128],
                                 xt_ap(0, dc, 0, 512),
                                 start=(dc == 0), stop=(dc == DC - 1))
                nc.tensor.matmul(kq0s[:, 512:1024],
                                 wq_sb[:, dc * 128:(dc + 1) * 128],
                                 xt_ap(0, dc, 0, 512),
                                 start=(dc == 0), stop=(dc == DC - 1))
                nc.tensor.matmul(v0ps[:],
                                 wv_sb[:, dc * 128:(dc + 1) * 128],
                                 xt_ap(0, dc, 0, 512),
                                 start=(dc == 0), stop=(dc == DC - 1))
            nc.vector.tensor_scalar_add(k2[0][:, 0:512], kq0s[:, 0:512],
                                        bk_sb[:])
            nc.vector.tensor_scalar_add(q2[0][:, 0:512], kq0s[:, 512:1024],
                                        bq_sb[:])
            nc.vector.tensor_scalar_add(vt[0][:, 0:512], v0ps[:], bv_sb[:])

        def transpose_v(b, nb):
            """Rotate vT[dh, n] -> v[n, dh] for one 128-row chunk, both heads
            in one PE transpose."""
            tp = tp_ps.tile([128, 128], bf16, name="tp", tag="tp")
            nc.tensor.transpose(
                tp[:], vt[b][:, nb * 128:(nb + 1) * 128], eye_sb[:])
            nc.vector.tensor_copy(
                v_sb[b][:, nb * 192:nb * 192 + 64], tp[:, 0:64])
            nc.vector.tensor_copy(
                v_sb[b][:, nb * 192 + 128:nb * 192 + 192], tp[:, 64:128])

        def emit_scores(b, j, ci, state):
            """Scores (both heads into one 2-bank psum tile) + exp for one
            128-m chunk."""
            t = ci - 4 * j
            lo = 128 * t if t >= 0 else 0
            sp = s_ps.tile([128, 1024], f32, name="s", tag="s")
            pt = pt_pool.tile([128, 1024], bf16, name="pt", tag="pt")
            for h in range(HL):
                hp = 64 * h
                nc.tensor.matmul(
                    sp[:, 512 * h + lo:512 * h + 512],
                    q2[b][hp:hp + 64, ci * 128:(ci + 1) * 128],
                    k2[b][hp:hp + 64, j * 512 + lo:(j + 1) * 512],
                    start=True, stop=True)
            if t < 0:
                nc.scalar.activation(pt[:], sp[:], Exp, scale=0.125)
            else:
                for h in range(HL):
                    nc.scalar.activation(pt[:, 512 * h + lo:512 * h + 512],
                                         sp[:, 512 * h + lo:512 * h + 512],
                                         Exp, scale=0.125)
                for h in range(HL):
                    nc.gpsimd.tensor_tensor(
                        pt[:, 512 * h + lo:512 * h + lo + 128],
                        pt[:, 512 * h + lo:512 * h + lo + 128],
                        m01_sb[:], mybir.AluOpType.mult)
            state[ci] = (pt, lo)

        def emit_pv(b, j, ci, pv, state, nch):
            pt, lo = state.pop(ci)
            for h in range(HL):
                nc.tensor.matmul(
                    pv[h][:, lo:512],
                    v_sb[b][:, ci * 192 + 64 * h:ci * 192 + 64 * h + 128],
                    pt[:, 512 * h + lo:512 * h + 512],
                    start=(ci == 0), stop=(ci == nch - 1))

        def att_norm(b, j, pv):
            for h in range(HL):
                # h0: psum rows 0:64 = sa, 64:128 = denom ; h1: swapped
                sa_rows = pv[h][64 * h:64 * h + 64, :]
                den_rows = pv[h][64 - 64 * h:128 - 64 * h, :]
                # denominators are sums of exp() in [~2e-3, ~3e3]: inside
                # approx_fast's domain; 18-bit accuracy is far below the bf16
                # noise of the P*V numerator. (approx_fast misreads PSUM
                # operands on HW - bounce through SBUF first.)
                den_sb = rc_pool.tile([64, 512], f32, name="den", tag="den")
                nc.scalar.copy(den_sb[:], den_rows)
                rc = rc_pool.tile([64, 512], f32, name="rc", tag="rc")
                nc.vector.reciprocal_approx_fast(rc[:], den_sb[:])
                nc.vector.tensor_tensor(
                    sa_sb[b][64 * h:64 * h + 64, j * 512:(j + 1) * 512],
                    sa_rows, rc[:], mybir.AluOpType.mult)

        def att_j(b, j, weave=(), tp_delay=0):
            """One attention j-block, software-pipelined: scores/exp of chunk
            ci+1 are emitted before PV of chunk ci; weave slots fill PE gaps;
            v-rotations for the block's own rows are spread over early
            chunks."""
            pv = [pv_ps.tile([128, 512], f32, name=f"pv{h}", tag="pv")
                  for h in range(HL)]
            weave = list(weave)
            nch = 4 * (j + 1)
            state = {}
            emitted = 0
            emit_scores(b, j, 0, state)
            for ci in range(nch):
                if ci + 1 < nch:
                    emit_scores(b, j, ci + 1, state)
                target = len(weave) * (ci + 1) // nch
                while emitted < target:
                    weave[emitted]()
                    emitted += 1
                if tp_delay <= ci < tp_delay + 4:
                    transpose_v(b, 4 * j + (ci - tp_delay))
                emit_pv(b, j, ci, pv, state, nch)
            att_norm(b, j, pv)

        # ================= emission schedule =================
        def ksl(b, nj):
            return make_proj_slots(b, 0, nj)

        def qsl(b, nj):
            return make_proj_slots(b, 1, nj)

        def vsl(b, nj):
            return make_proj_slots(b, 2, nj)

        def ops(b, nbs):
            return [make_op_slot(b, nb) for nb in nbs]

        kqv0_start()
        att_j(0, 0, weave=ksl(0, 1) + qsl(0, 1))
        att_j(0, 1, weave=vsl(0, 1) + ksl(0, 2) + qsl(0, 2) + vsl(0, 2),
              tp_delay=2)
        att_j(0, 2, weave=ksl(0, 3) + qsl(0, 3) + vsl(0, 3)
              + ksl(1, 0) + qsl(1, 0) + vsl(1, 0))
        att_j(0, 3, weave=ksl(1, 1) + qsl(1, 1) + vsl(1, 1)
              + ksl(1, 2) + qsl(1, 2) + vsl(1, 2) + ops(0, range(0, 4)))
        att_j(1, 0, weave=ksl(1, 3) + qsl(1, 3))
        att_j(1, 1, weave=vsl(1, 3) + ops(0, range(4, 8)))
        att_j(1, 2, weave=ops(0, range(8, 16)) + ops(1, range(0, 4)))
        att_j(1, 3, weave=ops(1, range(4, 12)))
        for nb in range(12, 16):
            make_op_slot(1, nb)()

    nc.compile()
    return nc


def _get_nc():
    if "nc" not in _CACHE:
        _CACHE["nc"] = _build()
    return _CACHE["nc"]


def _prep_inputs(x, Wkqv, bkqv, Wo, bo):
    """Host-side shard prep: one input map per core."""
    xt = np.ascontiguousarray(x.transpose(0, 2, 1)).astype(BF16)
    tri = np.triu(np.ones((128, 128), np.float32)).astype(BF16)  # m' <= n''
    eye2 = np.eye(128, dtype=np.float32).astype(BF16)
    in_maps = []
    for c in range(NCORES):
        h0, h1 = HL * c, HL * c + 1
        def shuf(w):
            # [D, 128] -> [128, DC*128]: partition = within-chunk row
            return np.ascontiguousarray(
                w.reshape(DC, 128, 128).transpose(1, 0, 2).reshape(128, DC * 128))

        wk2 = shuf(np.concatenate([Wkqv[h0, :, 0:64], Wkqv[h1, :, 0:64]], axis=1))
        wq2 = shuf(np.concatenate([Wkqv[h0, :, 64:128], Wkqv[h1, :, 64:128]], axis=1))
        wv2 = shuf(np.concatenate([Wkqv[h0, :, 128:192], Wkqv[h1, :, 128:192]], axis=1))
        bk2 = np.concatenate([bkqv[h0, 0:64], bkqv[h1, 0:64]])[:, None]
        bq2 = np.concatenate([bkqv[h0, 64:128], bkqv[h1, 64:128]])[:, None]
        bv2 = np.concatenate([bkqv[h0, 128:192], bkqv[h1, 128:192]])[:, None]
        in_maps.append({
            "xt": xt,
            "wk2": wk2.astype(BF16),
            "wq2": wq2.astype(BF16),
            "wv2": wv2.astype(BF16),
            "wo2": Wo[128 * c:128 * (c + 1), :].astype(BF16),
            "bk2": np.ascontiguousarray(bk2, np.float32),
            "bq2": np.ascontiguousarray(bq2, np.float32),
            "bv2": np.ascontiguousarray(bv2, np.float32),
            "eye2": eye2,
            "m01": tri,
        })
    return in_maps


def kernel(x, Wkqv, bkqv, Wo, bo):
    from concourse import bass_utils

    nc = _get_nc()
    in_maps = _prep_inputs(np.asarray(x), np.asarray(Wkqv), np.asarray(bkqv),
                           np.asarray(Wo), np.asarray(bo))
    res = bass_utils.run_bass_kernel_spmd(nc, in_maps, core_ids=list(range(NCORES)))
    acc = np.zeros((B, N, D), np.float32)
    for c in range(NCORES):
        acc += np.asarray(res.results[c]["out"], dtype=np.float32)
    acc += np.asarray(bo)[None, None, :]
    return acc


# revision 6
# speedup vs baseline: 1.3243x; 1.2437x over previous
"""Causal self-attention (k/q swapped variant) on 8 Trainium2 NeuronCores.

Problem (hardcoded shapes): B=2, N=2048, D=1024, H=16, DH=64.
  kqv = einsum('bnd,hde->bhne', x, Wkqv) + bkqv   ; split -> k, q, v
  A[b,h,n,m] = k[b,h,n]·q[b,h,m] / sqrt(DH), causal mask m<=n, softmax over m
  sa = A @ v ; concat heads ; out = sa @ Wo + bo

Sharding: batch x heads — core c owns batch c//4 and heads 4*(c%4)..+4 (two
head-pairs A/B), computes its partial output projection sa_local @ Wo[rows]
in bf16 over its single batch, and the host sums 4 partials per batch (+bo)
in fp32. This halves both the x input DMA and the partial-output DMA vs
all-batches-per-core head sharding.

Per-core device kernel (all matmul operands bf16, fp32 PSUM accumulation):
  - x is pre-transposed on host to xt = x[b].T ([D, N]) so the contraction
    dim d lands on SBUF partitions; one copy shared by both head-pairs.
  - scores are computed transposed, S^T[m, n] = q[m]·k[n], so softmax's
    reduction dim m sits on partitions; both heads of a pair live in ONE
    [128, 1024] PSUM tile (2 banks) so off-diagonal chunks need a single wide
    exp() on the ACT engine. The denominator comes free from the PV matmul by
    augmenting v with 64 ones columns.
  - the chunk loop is software-pipelined: scores+exp of chunk ci+1 are
    emitted before the PV of chunk ci; projection / output-projection work is
    woven between chunks in ~0.9us slots to fill the remaining PE gaps.
  - output projection accumulates both pairs' contributions on device; the
    two psum halves of a block alternate over two banks so the next block's
    matmuls overlap the previous block's psum->bf16 cast.
  - PSUM budget (8 banks): scores 2x[128,1024] (4) + PV accumulators 2 +
    proj/outproj bank 1 + transpose/outproj bank 1.
"""

import numpy as np
import ml_dtypes

B = 2
N = 2048
D = 1024
H = 16
DH = 64
NCORES = 8
HL = 2                    # heads per pair
NPAIR = 2                 # head-pairs per core
DC = D // 128             # contraction chunks = 8
NB = N // 128             # 128-row blocks = 16
NJ = N // 512             # 512-col blocks = 4

BF16 = ml_dtypes.bfloat16

_CACHE = {}


def _build():
    import concourse.bass as bass
    import concourse.mybir as mybir
    import concourse.tile as tile
    from concourse import bacc
    from contextlib import ExitStack

    f32 = mybir.dt.float32
    bf16 = mybir.dt.bfloat16
    Exp = mybir.ActivationFunctionType.Exp

    nc = bacc.Bacc("TRN2", target_bir_lowering=False, debug=False,
                   enable_asserts=False, num_devices=NCORES)

    xt_d = nc.dram_tensor("xt", [D, N], bf16, kind="ExternalInput")
    # k/q/v weights arrive pre-shuffled to the SBUF layout [128, DC*128]
    # (partition = within-chunk row, free = (chunk, head-col)), per pair
    wk_d = {p: nc.dram_tensor(f"wk{p}", [128, DC * 128], bf16,
                              kind="ExternalInput") for p in range(NPAIR)}
    wq_d = {p: nc.dram_tensor(f"wq{p}", [128, DC * 128], bf16,
                              kind="ExternalInput") for p in range(NPAIR)}
    wv_d = {p: nc.dram_tensor(f"wv{p}", [128, DC * 128], bf16,
                              kind="ExternalInput") for p in range(NPAIR)}
    wo_d = {p: nc.dram_tensor(f"wo{p}", [128, D], bf16,
                              kind="ExternalInput") for p in range(NPAIR)}
    bk_d = {p: nc.dram_tensor(f"bk{p}", [128, 1], f32,
                              kind="ExternalInput") for p in range(NPAIR)}
    bq_d = {p: nc.dram_tensor(f"bq{p}", [128, 1], f32,
                              kind="ExternalInput") for p in range(NPAIR)}
    bv_d = {p: nc.dram_tensor(f"bv{p}", [128, 1], f32,
                              kind="ExternalInput") for p in range(NPAIR)}
    eye_d = nc.dram_tensor("eye2", [128, 128], bf16, kind="ExternalInput")
    m01_d = nc.dram_tensor("m01", [128, 128], bf16, kind="ExternalInput")
    out_d = nc.dram_tensor("out", [N, D], bf16, kind="ExternalOutput")

    with tile.TileContext(nc) as tc, ExitStack() as ctx:
        const = ctx.enter_context(tc.tile_pool(name="const", bufs=1))
        xt_pool = ctx.enter_context(tc.tile_pool(name="xt", bufs=1))
        kq_pool = ctx.enter_context(tc.tile_pool(name="kq", bufs=6))
        v_pool = ctx.enter_context(tc.tile_pool(name="v", bufs=2))
        sa_pool = ctx.enter_context(tc.tile_pool(name="sa", bufs=2))
        pt_pool = ctx.enter_context(tc.tile_pool(name="pt", bufs=6))
        rc_pool = ctx.enter_context(tc.tile_pool(name="rc", bufs=2))
        ob_pool = ctx.enter_context(tc.tile_pool(name="ob", bufs=4))
        s_ps = ctx.enter_context(tc.tile_pool(name="s_ps", bufs=2, space="PSUM"))
        pv_ps = ctx.enter_context(tc.tile_pool(name="pv_ps", bufs=2, space="PSUM"))
        wv_ps = ctx.enter_context(tc.tile_pool(name="wv_ps", bufs=1, space="PSUM"))
        tp_ps = ctx.enter_context(tc.tile_pool(name="tp_ps", bufs=1, space="PSUM"))

        # ---- DMA issue order: pair-A weights -> xt half-0 pieces (paced,
        # two rings) -> small consts -> xt half-1 pieces -> pair-B weights.
        wk_sb, wq_sb, wv_sb, wo_sb = {}, {}, {}, {}
        bk_sb, bq_sb, bv_sb = {}, {}, {}
        for p in range(NPAIR):
            wk_sb[p] = const.tile([128, DC * 128], bf16, name=f"wk_sb{p}")
            wq_sb[p] = const.tile([128, DC * 128], bf16, name=f"wq_sb{p}")
            wv_sb[p] = const.tile([128, DC * 128], bf16, name=f"wv_sb{p}")
            wo_sb[p] = const.tile([128, D], bf16, name=f"wo_sb{p}")
            bk_sb[p] = const.tile([128, 1], f32, name=f"bk_sb{p}")
            bq_sb[p] = const.tile([128, 1], f32, name=f"bq_sb{p}")
            bv_sb[p] = const.tile([128, 1], f32, name=f"bv_sb{p}")
        eye_sb = const.tile([128, 128], bf16, name="eye_sb")
        m01_sb = const.tile([128, 128], bf16, name="m01_sb")

        nc.sync.dma_start(wk_sb[0][:], wk_d[0].ap())
        nc.sync.dma_start(wq_sb[0][:], wq_d[0].ap())
        nc.sync.dma_start(wv_sb[0][:], wv_d[0].ap())
        xt0 = {}   # (dc, half) -> [128, 1024]
        for half in range(2):
            for dc in range(DC):
                t = xt_pool.tile([128, 1024], bf16, name=f"xt_{dc}_{half}",
                                 tag="xt", bufs=2 * DC)
                eng = nc.sync if dc % 2 == 0 else nc.scalar
                eng.dma_start(t[:], xt_d.ap()[dc * 128:(dc + 1) * 128,
                                              half * 1024:(half + 1) * 1024])
                xt0[dc, half] = t
            if half == 0:
                nc.sync.dma_start(eye_sb[:], eye_d.ap())
                nc.sync.dma_start(m01_sb[:], m01_d.ap())
                for p in range(NPAIR):
                    nc.sync.dma_start(bk_sb[p][:], bk_d[p].ap())
                    nc.sync.dma_start(bq_sb[p][:], bq_d[p].ap())
                    nc.sync.dma_start(bv_sb[p][:], bv_d[p].ap())
        nc.sync.dma_start(wk_sb[1][:], wk_d[1].ap())
        nc.sync.dma_start(wq_sb[1][:], wq_d[1].ap())
        nc.sync.dma_start(wv_sb[1][:], wv_d[1].ap())
        nc.sync.dma_start(wo_sb[0][:], wo_d[0].ap())
        nc.sync.dma_start(wo_sb[1][:], wo_d[1].ap())

        def xt_ap(dc, c0, c1):
            half = c0 // 1024
            return xt0[dc, half][:, c0 - half * 1024:c1 - half * 1024]

        # ---- per-pair tensors
        k2 = {p: kq_pool.tile([128, N], bf16, name=f"k2_p{p}", tag="kq")
              for p in range(NPAIR)}
        q2 = {p: kq_pool.tile([128, N], bf16, name=f"q2_p{p}", tag="kq")
              for p in range(NPAIR)}
        vt = {p: kq_pool.tile([128, N], bf16, name=f"vt_p{p}", tag="kq")
              for p in range(NPAIR)}
        v_sb = {}
        sa_sb = {}
        groups = {p: ((wk_sb[p], bk_sb[p], k2[p]), (wq_sb[p], bq_sb[p], q2[p]),
                      (wv_sb[p], bv_sb[p], vt[p])) for p in range(NPAIR)}

        for p in range(NPAIR):
            v_sb[p] = v_pool.tile([128, NB * 192], bf16, name=f"v_p{p}",
                                  tag="v")
            sa_sb[p] = sa_pool.tile([128, N], bf16, name=f"sa_p{p}", tag="sa")
            nc.gpsimd.memset(
                v_sb[p][:].rearrange("p (nb g) -> p nb g", g=192)[:, :, 64:128],
                1.0)

        def make_proj_slots(p, gi, nj):
            """One [128, 512] projection group as two weave slots (dc 0-3 and
            dc 4-7 + bias add), sharing the wv PSUM bank."""
            st = {}

            def part(d0, d1):
                def go():
                    if d0 == 0:
                        st["ps"] = wv_ps.tile([128, 512], f32, name="wvps",
                                              tag="wv")
                    ps = st["ps"]
                    w_sb, bias_sb, dst = groups[p][gi]
                    for dc in range(d0, d1):
                        nc.tensor.matmul(
                            ps[:], w_sb[:, dc * 128:(dc + 1) * 128],
                            xt_ap(dc, nj * 512, (nj + 1) * 512),
                            start=(dc == 0), stop=(dc == DC - 1))
                    if d1 == DC:
                        nc.vector.tensor_scalar_add(
                            dst[:, nj * 512:(nj + 1) * 512], ps[:], bias_sb[:])
                return go
            return [part(0, 4), part(4, DC)]

        def make_op_slot(nb, cast_eng=None):
            """Output projection of one 128-row block: both pairs accumulate;
            the two column halves alternate over the wv / tp banks so casts
            overlap the next matmuls; one [128, 1024] bf16 DMA per block."""
            def go():
                ob = ob_pool.tile([128, 1024], bf16, name="ob", tag="ob")
                for half in range(2):
                    pool, tag = (wv_ps, "wv") if half == 0 else (tp_ps, "tp")
                    op = pool.tile([128, 512], f32, name="opps", tag=tag,
                                   padded_shape=[128, 512])
                    for p in range(NPAIR):
                        nc.tensor.matmul(
                            op[:], sa_sb[p][:, nb * 128:(nb + 1) * 128],
                            wo_sb[p][:, half * 512:(half + 1) * 512],
                            start=(p == 0), stop=(p == NPAIR - 1))
                    eng = cast_eng or nc.vector
                    if eng is nc.scalar:
                        eng.copy(ob[:, half * 512:(half + 1) * 512], op[:])
                    else:
                        eng.tensor_copy(ob[:, half * 512:(half + 1) * 512],
                                        op[:])
                nc.sync.dma_start(out_d.ap()[nb * 128:(nb + 1) * 128, :],
                                  ob[:])
            return go

        def kqv0_start():
            """Pair-A k/q/v projections for nj=0, d-chunk-major so the PE is
            paced by the half-0 xt piece DMAs (k/q in one score-pool tile)."""
            kq0s = s_ps.tile([128, 1024], f32, name="kq0s", tag="s")
            v0ps = wv_ps.tile([128, 512], f32, name="v0ps", tag="wv")
            for dc in range(DC):
                nc.tensor.matmul(kq0s[:, 0:512],
                                 wk_sb[0][:, dc * 128:(dc + 1) * 128],
                                 xt_ap(dc, 0, 512),
                                 start=(dc == 0), stop=(dc == DC - 1))
                nc.tensor.matmul(kq0s[:, 512:1024],
                                 wq_sb[0][:, dc * 128:(dc + 1) * 128],
                                 xt_ap(dc, 0, 512),
                                 start=(dc == 0), stop=(dc == DC - 1))
                nc.tensor.matmul(v0ps[:],
                                 wv_sb[0][:, dc * 128:(dc + 1) * 128],
                                 xt_ap(dc, 0, 512),
                                 start=(dc == 0), stop=(dc == DC - 1))
            nc.vector.tensor_scalar_add(k2[0][:, 0:512], kq0s[:, 0:512],
                                        bk_sb[0][:])
            nc.vector.tensor_scalar_add(q2[0][:, 0:512], kq0s[:, 512:1024],
                                        bq_sb[0][:])
            nc.vector.tensor_scalar_add(vt[0][:, 0:512], v0ps[:], bv_sb[0][:])

        def transpose_v(p, nb):
            """Rotate vT[dh, n] -> v[n, dh] for one 128-row chunk, both heads
            in one PE transpose."""
            tp = tp_ps.tile([128, 128], bf16, name="tp", tag="tp")
            nc.tensor.transpose(
                tp[:], vt[p][:, nb * 128:(nb + 1) * 128], eye_sb[:])
            nc.vector.tensor_copy(
                v_sb[p][:, nb * 192:nb * 192 + 64], tp[:, 0:64])
            nc.vector.tensor_copy(
                v_sb[p][:, nb * 192 + 128:nb * 192 + 192], tp[:, 64:128])

        def emit_scores(p, j, ci, state):
            """Scores (both heads into one 2-bank psum tile) + exp for one
            128-m chunk."""
            t = ci - 4 * j
            lo = 128 * t if t >= 0 else 0
            sp = s_ps.tile([128, 1024], f32, name="s", tag="s")
            pt = pt_pool.tile([128, 1024], bf16, name="pt", tag="pt")
            for h in range(HL):
                hp = 64 * h
                nc.tensor.matmul(
                    sp[:, 512 * h + lo:512 * h + 512],
                    q2[p][hp:hp + 64, ci * 128:(ci + 1) * 128],
                    k2[p][hp:hp + 64, j * 512 + lo:(j + 1) * 512],
                    start=True, stop=True)
            if t < 0:
                nc.scalar.activation(pt[:], sp[:], Exp, scale=0.125)
            else:
                for h in range(HL):
                    nc.scalar.activation(pt[:, 512 * h + lo:512 * h + 512],
                                         sp[:, 512 * h + lo:512 * h + 512],
                                         Exp, scale=0.125)
                for h in range(HL):
                    nc.vector.tensor_tensor(
                        pt[:, 512 * h + lo:512 * h + lo + 128],
                        pt[:, 512 * h + lo:512 * h + lo + 128],
                        m01_sb[:], mybir.AluOpType.mult)
            state[ci] = (pt, lo)

        def emit_pv(p, j, ci, pv, state, nch):
            pt, lo = state.pop(ci)
            for h in range(HL):
                nc.tensor.matmul(
                    pv[h][:, lo:512],
                    v_sb[p][:, ci * 192 + 64 * h:ci * 192 + 64 * h + 128],
                    pt[:, 512 * h + lo:512 * h + 512],
                    start=(ci == 0), stop=(ci == nch - 1))

        def att_norm(p, j, pv):
            for h in range(HL):
                # h0: psum rows 0:64 = sa, 64:128 = denom ; h1: swapped
                sa_rows = pv[h][64 * h:64 * h + 64, :]
                den_rows = pv[h][64 - 64 * h:128 - 64 * h, :]
                # denominators are sums of exp() in [~2e-3, ~3e3]: inside
                # approx_fast's domain; 18-bit accuracy is far below the bf16
                # noise of the P*V numerator. (approx_fast misreads PSUM
                # operands on HW - bounce through SBUF first.)
                den_sb = rc_pool.tile([64, 512], f32, name="den", tag="den")
                nc.scalar.copy(den_sb[:], den_rows)
                rc = rc_pool.tile([64, 512], f32, name="rc", tag="rc")
                nc.vector.reciprocal_approx_fast(rc[:], den_sb[:])
                nc.vector.tensor_tensor(
                    sa_sb[p][64 * h:64 * h + 64, j * 512:(j + 1) * 512],
                    sa_rows, rc[:], mybir.AluOpType.mult)

        def att_j(p, j, weave=(), tp_delay=0):
            """One attention j-block, software-pipelined: scores/exp of chunk
            ci+1 are emitted before PV of chunk ci; weave slots fill PE gaps;
            v-rotations for the block's own rows are spread over early
            chunks."""
            pv = [pv_ps.tile([128, 512], f32, name=f"pv{h}", tag="pv")
                  for h in range(HL)]
            weave = list(weave)
            nch = 4 * (j + 1)
            state = {}
            emitted = 0
            emit_scores(p, j, 0, state)
            for ci in range(nch):
                if ci + 1 < nch:
                    emit_scores(p, j, ci + 1, state)
                target = len(weave) * (ci + 1) // nch
                while emitted < target:
                    weave[emitted]()
                    emitted += 1
                if tp_delay <= ci < tp_delay + 4:
                    transpose_v(p, 4 * j + (ci - tp_delay))
                emit_pv(p, j, ci, pv, state, nch)
            att_norm(p, j, pv)

        # ================= emission schedule =================
        def ksl(p, nj):
            return make_proj_slots(p, 0, nj)

        def qsl(p, nj):
            return make_proj_slots(p, 1, nj)

        def vsl(p, nj):
            return make_proj_slots(p, 2, nj)

        def ops(nbs):
            return [make_op_slot(nb) for nb in nbs]

        kqv0_start()
        att_j(0, 0, weave=ksl(0, 1) + qsl(0, 1))
        att_j(0, 1, weave=vsl(0, 1) + ksl(0, 2) + qsl(0, 2) + vsl(0, 2),
              tp_delay=2)
        att_j(0, 2, weave=ksl(0, 3) + qsl(0, 3) + vsl(0, 3)
              + ksl(1, 0) + qsl(1, 0) + vsl(1, 0))
        att_j(0, 3, weave=ksl(1, 1) + qsl(1, 1) + vsl(1, 1)
              + ksl(1, 2) + qsl(1, 2) + vsl(1, 2))
        att_j(1, 0, weave=ksl(1, 3) + qsl(1, 3))
        att_j(1, 1, weave=vsl(1, 3) + ops(range(0, 4)))
        att_j(1, 2, weave=ops(range(4, 8)))
        att_j(1, 3, weave=ops(range(8, 12)))
        for nb in range(12, 16):
            make_op_slot(nb, cast_eng=nc.scalar)()

    nc.compile()
    return nc


def _get_nc():
    if "nc" not in _CACHE:
        _CACHE["nc"] = _build()
    return _CACHE["nc"]


def _prep_inputs(x, Wkqv, bkqv, Wo, bo):
    """Host-side shard prep: one input map per core (core c: batch c//4,
    head-pairs (4*(c%4), 4*(c%4)+1) and (+2, +3))."""
    tri = np.triu(np.ones((128, 128), np.float32)).astype(BF16)  # m' <= n''
    eye2 = np.eye(128, dtype=np.float32).astype(BF16)
    xts = [np.ascontiguousarray(x[b].T).astype(BF16) for b in range(B)]

    def shuf(w):
        # [D, 128] -> [128, DC*128]: partition = within-chunk row
        return np.ascontiguousarray(
            w.reshape(DC, 128, 128).transpose(1, 0, 2).reshape(128, DC * 128))

    in_maps = []
    for c in range(NCORES):
        b, m = c // 4, c % 4
        im = {"xt": xts[b], "eye2": eye2, "m01": tri}
        for p in range(NPAIR):
            h0 = 4 * m + 2 * p
            h1 = h0 + 1
            im[f"wk{p}"] = shuf(np.concatenate(
                [Wkqv[h0, :, 0:64], Wkqv[h1, :, 0:64]], axis=1)).astype(BF16)
            im[f"wq{p}"] = shuf(np.concatenate(
                [Wkqv[h0, :, 64:128], Wkqv[h1, :, 64:128]], axis=1)).astype(BF16)
            im[f"wv{p}"] = shuf(np.concatenate(
                [Wkqv[h0, :, 128:192], Wkqv[h1, :, 128:192]], axis=1)).astype(BF16)
            im[f"wo{p}"] = Wo[64 * h0:64 * h0 + 128, :].astype(BF16)
            im[f"bk{p}"] = np.ascontiguousarray(np.concatenate(
                [bkqv[h0, 0:64], bkqv[h1, 0:64]])[:, None], np.float32)
            im[f"bq{p}"] = np.ascontiguousarray(np.concatenate(
                [bkqv[h0, 64:128], bkqv[h1, 64:128]])[:, None], np.float32)
            im[f"bv{p}"] = np.ascontiguousarray(np.concatenate(
                [bkqv[h0, 128:192], bkqv[h1, 128:192]])[:, None], np.float32)
        in_maps.append(im)
    return in_maps


def kernel(x, Wkqv, bkqv, Wo, bo):
    from concourse import bass_utils

    nc = _get_nc()
    in_maps = _prep_inputs(np.asarray(x), np.asarray(Wkqv), np.asarray(bkqv),
                           np.asarray(Wo), np.asarray(bo))
    res = bass_utils.run_bass_kernel_spmd(nc, in_maps, core_ids=list(range(NCORES)))
    acc = np.zeros((B, N, D), np.float32)
    for c in range(NCORES):
        acc[c // 4] += np.asarray(res.results[c]["out"], dtype=np.float32)
    acc += np.asarray(bo)[None, None, :]
    return acc
